# revision 15
# baseline (speedup 1.0000x reference)
"""Trainium2 Bass kernel for nn_ClusterLM (vq_codebook).

Computes, for embeddings E [65536, 768] and centroids C [256, 768]:
  dist[b,k]  = || E_b - C_k ||_2
  labels[b]  = argmin_k dist[b,k]
  loss       = mean_k( sum_b dist[b,k] * softmin_b(dist[:,k]) )

Data-parallel over 8 NeuronCores: each core handles 8192 rows of E, with
C replicated.  Per-centroid softmin statistics use a fixed exp-shift
(exp(SHIFT - dist)) so the per-core partial sums S_k = sum_b exp(SHIFT-d)
and W_k = sum_b d*exp(SHIFT-d) merge on the host with no collective.

Layout on core: [b, k] (batch rows on partitions, centroids on free dim).
  - E tiles [128, 768] are PE-transposed to eT [d, b] chunks for the
    d-contraction matmuls (out[b,k] = sum_d eT[d,b] * cT[d,k]).
  - cT holds 2*C.T so psum = 2*dot; t = 2*dot - c2 makes argmax_k(t) the
    argmin over distances, and dist = sqrt(e2 - t).
  - S/W column sums over the batch go through ones.T @ [ex|w] matmuls.
"""

import os
import sys

sys.path.insert(0, "/opt/trn_rl_repo")

from contextlib import ExitStack

import numpy as np

import concourse.bacc as bacc
import concourse.bass as bass
import concourse.mybir as mybir
import concourse.tile as tile
from concourse import masks
from concourse.bass_utils import run_bass_kernel_spmd

F32 = mybir.dt.float32
F32R = mybir.dt.float32r
BF16 = mybir.dt.bfloat16
I32 = mybir.dt.int32
U32 = mybir.dt.uint32
AF = mybir.ActivationFunctionType
ALU = mybir.AluOpType

P = 128          # partitions
D = 768          # embedding dim
K = 256          # centroids
NCHUNK = D // P  # 6 contraction chunks
N_CORES = 8
B = 65536
B_CORE = B // N_CORES       # 8192
TILES = B_CORE // P         # 64
GROUP = 8                   # tiles per epilogue group
NGROUPS = TILES // GROUP    # 8
SHIFT = 36.0                # exp(SHIFT - dist); dist in [33, 45] for randn data

USE_F32R = os.environ.get("VQ_USE_F32R", "1") == "1"
E2_DVE_EVERY = int(os.environ.get("VQ_E2_DVE_EVERY", "3"))  # 0=all ACT; n: every n-th tile on DVE
W_ON_GPSIMD = os.environ.get("VQ_W_GPS", "1") == "1"


def build_kernel(use_f32r: bool):
    mm_dt = F32R if use_f32r else F32
    nc = bacc.Bacc("TRN2", target_bir_lowering=False, debug=False)

    emb = nc.dram_tensor("embeddings", [B_CORE, D], F32, kind="ExternalInput").ap()
    cen = nc.dram_tensor("centroids", [K, D], F32, kind="ExternalInput").ap()
    labels_out = nc.dram_tensor("labels", [TILES, P], I32, kind="ExternalOutput").ap()
    sums_out = nc.dram_tensor("sums", [1, 2 * K], F32, kind="ExternalOutput").ap()

    with tile.TileContext(nc) as tc, ExitStack() as ctx:
        const_pool = ctx.enter_context(tc.tile_pool(name="const", bufs=1))
        epool = ctx.enter_context(tc.tile_pool(name="e", bufs=2))
        etpool = ctx.enter_context(tc.tile_pool(name="et", bufs=GROUP + 2))
        sqpool = ctx.enter_context(tc.tile_pool(name="sq", bufs=2))
        e2pool = ctx.enter_context(tc.tile_pool(name="e2", bufs=6))
        tpool = ctx.enter_context(tc.tile_pool(name="t", bufs=2))
        distpool = ctx.enter_context(tc.tile_pool(name="dist", bufs=2))
        exwpool = ctx.enter_context(tc.tile_pool(name="exw", bufs=3))
        m8pool = ctx.enter_context(tc.tile_pool(name="m8", bufs=6))
        outpool = ctx.enter_context(tc.tile_pool(name="out", bufs=1))

        ps_et = ctx.enter_context(tc.tile_pool(name="ps_et", bufs=2, space="PSUM"))
        ps_dot = ctx.enter_context(tc.tile_pool(name="ps_dot", bufs=2, space="PSUM"))
        ps_sums = ctx.enter_context(tc.tile_pool(name="ps_sums", bufs=1, space="PSUM"))

        # ---------------- constants / centroid prep ----------------
        ident = const_pool.tile([P, P], F32)
        masks.make_identity(nc, ident[:])

        ones_bf = const_pool.tile([P, 1], BF16)
        nc.vector.memset(ones_bf[:], 1.0)
        shift_sb = const_pool.tile([P, 1], F32)
        nc.vector.memset(shift_sb[:], SHIFT)
        ones128 = const_pool.tile([P, P], F32)
        nc.vector.memset(ones128[:], 1.0)

        c_nat = const_pool.tile([P, 2, D], F32)  # two 128-row halves of C
        nc.sync.dma_start(c_nat[:], cen.rearrange("(h p) d -> p h d", h=2))

        # cT chunks: [d, k] with k = 256 across both halves
        ctsc = const_pool.tile([P, NCHUNK, K], mm_dt)   # 2 * C.T (rounded)
        ctsq = const_pool.tile([P, NCHUNK, K], F32)     # (C.T)^2
        for c in range(NCHUNK):
            pct = ps_dot.tile([P, 512], F32, tag="pd")
            for h in range(2):
                nc.tensor.transpose(
                    pct[:, h * P:(h + 1) * P],
                    c_nat[:, h, c * P:(c + 1) * P],
                    ident[:],
                )
            nc.scalar.activation(ctsc[:, c, :], pct[:, 0:K], AF.Copy, scale=2.0)
            nc.scalar.activation(ctsq[:, c, :], pct[:, 0:K], AF.Square)

        # c2 replicated across partitions: ones128.T @ ctsq  (fp32 matmuls)
        pc2 = ps_dot.tile([P, 512], F32, tag="pd")
        for c in range(NCHUNK):
            nc.tensor.matmul(pc2[:, 0:K], ones128[:], ctsq[:, c, :],
                             start=(c == 0), stop=(c == NCHUNK - 1))
        c2rep = const_pool.tile([P, K], F32)
        nc.vector.tensor_copy(c2rep[:], pc2[:, 0:K])

        labels_sb = const_pool.tile([P, TILES], F32)

        psum_s = ps_sums.tile([1, 2 * K], F32)

        emb_r = emb.rearrange("(g j p) d -> g p j d", g=NGROUPS, j=GROUP, p=P)

        # sums matmuls for group g are emitted mid-way through group g+1 so
        # the PE never waits on the sqrt->exp->w tail (software pipelining)
        def emit_sums(exw_prev, g_prev):
            for j in range(GROUP):
                idx = g_prev * GROUP + j
                nc.tensor.matmul(psum_s[:], ones_bf[:], exw_prev[:, j, :],
                                 start=(idx == 0), stop=(idx == TILES - 1),
                                 skip_group_check=True)

        pending = None

        # ---------------- main loop ----------------
        for g in range(NGROUPS):
            e_g = epool.tile([P, GROUP, D], F32)
            nc.sync.dma_start(e_g[:], emb_r[g])

            t_g = tpool.tile([P, GROUP, K], F32)
            dist_g = distpool.tile([P, GROUP, K], BF16)
            exw = exwpool.tile([P, GROUP, 2 * K], BF16)
            m8g = m8pool.tile([P, GROUP, 8], F32, tag="m8")
            i8g = m8pool.tile([P, GROUP, 8], U32, tag="i8")

            # ---- phase 1: e2 + transposes + PSUM->SBUF copies (batched so
            # the PE stays in transpose mode for the whole group) ----
            ets = []
            e2s = []
            for j in range(GROUP):
                ei = e_g[:, j, :]
                e2c = e2pool.tile([P, 1], F32)
                on_dve = E2_DVE_EVERY > 0 and j % E2_DVE_EVERY == E2_DVE_EVERY - 1
                if on_dve:
                    sqd = sqpool.tile([P, D], BF16, tag="sqv")
                    nc.vector.scalar_tensor_tensor(
                        out=sqd[:], in0=ei, scalar=1.0, in1=ei,
                        op0=ALU.mult, op1=ALU.mult, accum_out=e2c[:])
                else:
                    sqd = sqpool.tile([P, D], BF16, tag="sqa")
                    nc.scalar.activation(sqd[:], ei, AF.Square, accum_out=e2c[:])
                e2s.append(e2c)

                pet = ps_et.tile([P, D], F32)
                for c in range(NCHUNK):
                    nc.tensor.transpose(pet[:, c * P:(c + 1) * P],
                                        ei[:, c * P:(c + 1) * P], ident[:])
                et = etpool.tile([P, D], mm_dt)
                nc.scalar.copy(et[:, 0:512], pet[:, 0:512])
                nc.vector.tensor_copy(et[:, 512:D], pet[:, 512:D])
                ets.append(et)

            # ---- phase 2: dots + epilogue per tile ----
            pd = None
            for j in range(GROUP):
                et = ets[j]
                if j % 2 == 0:
                    pd = ps_dot.tile([P, 512], F32, tag="pd")
                half = (j % 2) * K
                for c in range(NCHUNK):
                    nc.tensor.matmul(pd[:, half:half + K],
                                     et[:, c * P:(c + 1) * P],
                                     ctsc[:, c, :],
                                     start=(c == 0), stop=(c == NCHUNK - 1))

                # t = (2*dot - e2) - c2 = -dist^2   (argmax_k t == argmin_k)
                tj = t_g[:, j, :]
                nc.vector.scalar_tensor_tensor(
                    out=tj, in0=pd[:, half:half + K], scalar=e2s[j][:],
                    in1=c2rep[:], op0=ALU.subtract, op1=ALU.subtract,
                )
                nc.vector.max(m8g[:, j, :], tj)
                nc.vector.max_index(i8g[:, j, :], m8g[:, j, :], tj)

                if j == 1 and pending is not None:
                    emit_sums(*pending)
                    pending = None

            # labels for the whole group in one cast  [P, GROUP]
            nc.vector.tensor_copy(
                labels_sb[:, g * GROUP:(g + 1) * GROUP],
                i8g[:, :, 0])

            # dist = sqrt(-t), batched over the group
            nc.scalar.activation(dist_g[:], t_g[:], AF.Sqrt, scale=-1.0)
            # ex = exp(SHIFT - dist)
            nc.scalar.activation(exw[:, :, 0:K], dist_g[:],
                                 AF.Exp, bias=shift_sb[:], scale=-1.0)
            # w = dist * ex
            w_eng = nc.gpsimd if W_ON_GPSIMD else nc.vector
            w_eng.tensor_tensor(out=exw[:, :, K:2 * K], in0=dist_g[:],
                                in1=exw[:, :, 0:K], op=ALU.mult)
            pending = (exw, g)

        emit_sums(*pending)

        # ---------------- outputs ----------------
        sums_sb = outpool.tile([1, 2 * K], F32)
        nc.vector.tensor_copy(sums_sb[:], psum_s[:])
        nc.sync.dma_start(sums_out[:], sums_sb[:])

        plab = ps_dot.tile([P, 512], F32, tag="pd")
        nc.tensor.transpose(plab[:TILES, 0:P], labels_sb[:], ident[:])
        lab_i32 = outpool.tile([TILES, P], I32)
        nc.vector.tensor_copy(lab_i32[:], plab[:TILES, 0:P])
        nc.sync.dma_start(labels_out[:], lab_i32[:])

    nc.compile()
    return nc


_NC_CACHE = {}


def _get_nc(use_f32r: bool):
    if use_f32r not in _NC_CACHE:
        _NC_CACHE[use_f32r] = build_kernel(use_f32r)
    return _NC_CACHE[use_f32r]


def kernel(embeddings, centroids, _trace=False, _use_f32r=None):
    use_f32r = USE_F32R if _use_f32r is None else _use_f32r
    embeddings = np.ascontiguousarray(np.asarray(embeddings, dtype=np.float32))
    centroids = np.ascontiguousarray(np.asarray(centroids, dtype=np.float32))
    assert embeddings.shape == (B, D) and centroids.shape == (K, D)

    nc = _get_nc(use_f32r)
    in_maps = [
        {
            "embeddings": embeddings[i * B_CORE:(i + 1) * B_CORE],
            "centroids": centroids,
        }
        for i in range(N_CORES)
    ]
    res = run_bass_kernel_spmd(nc, in_maps, core_ids=list(range(N_CORES)),
                               trace=_trace)

    labels = np.concatenate(
        [res.results[i]["labels"].reshape(-1) for i in range(N_CORES)]
    ).astype(np.int32)

    sums = np.stack([res.results[i]["sums"][0] for i in range(N_CORES)])  # [8, 512]
    S = sums[:, 0:K].sum(axis=0)
    W = sums[:, K:2 * K].sum(axis=0)
    loss = np.float32(np.mean(W / S))

    if _trace:
        kernel._last_results = res
    return loss, labels


# revision 18
# speedup vs baseline: 1.0010x; 1.0010x over previous
"""Trainium2 Bass kernel for nn_ClusterLM (vq_codebook).

Computes, for embeddings E [65536, 768] and centroids C [256, 768]:
  dist[b,k]  = || E_b - C_k ||_2
  labels[b]  = argmin_k dist[b,k]
  loss       = mean_k( sum_b dist[b,k] * softmin_b(dist[:,k]) )

Data-parallel over 8 NeuronCores: each core handles 8192 rows of E, with
C replicated.  Per-centroid softmin statistics use a fixed exp-shift
(exp(SHIFT - dist)) so the per-core partial sums S_k = sum_b exp(SHIFT-d)
and W_k = sum_b d*exp(SHIFT-d) merge on the host with no collective.

Layout on core: [b, k] (batch rows on partitions, centroids on free dim).
  - E tiles [128, 768] are PE-transposed to eT [d, b] chunks for the
    d-contraction matmuls (out[b,k] = sum_d eT[d,b] * cT[d,k]).
  - cT holds 2*C.T so psum = 2*dot; t = 2*dot - c2 makes argmax_k(t) the
    argmin over distances, and dist = sqrt(e2 - t).
  - S/W column sums over the batch go through ones.T @ [ex|w] matmuls.
"""

import os
import sys

sys.path.insert(0, "/opt/trn_rl_repo")

from contextlib import ExitStack

import numpy as np

import concourse.bacc as bacc
import concourse.bass as bass
import concourse.mybir as mybir
import concourse.tile as tile
from concourse import masks
from concourse.bass_utils import run_bass_kernel_spmd

F32 = mybir.dt.float32
F32R = mybir.dt.float32r
BF16 = mybir.dt.bfloat16
I32 = mybir.dt.int32
U32 = mybir.dt.uint32
AF = mybir.ActivationFunctionType
ALU = mybir.AluOpType

P = 128          # partitions
D = 768          # embedding dim
K = 256          # centroids
NCHUNK = D // P  # 6 contraction chunks
N_CORES = 8
B = 65536
B_CORE = B // N_CORES       # 8192
TILES = B_CORE // P         # 64
GROUP = 8                   # tiles per epilogue group
NGROUPS = TILES // GROUP    # 8
SHIFT = 36.0                # exp(SHIFT - dist); dist in [33, 45] for randn data

USE_F32R = os.environ.get("VQ_USE_F32R", "1") == "1"
E2_DVE_EVERY = int(os.environ.get("VQ_E2_DVE_EVERY", "3"))  # 0=all ACT; n: every n-th tile on DVE
W_ON_GPSIMD = os.environ.get("VQ_W_GPS", "1") == "1"


def build_kernel(use_f32r: bool):
    mm_dt = F32R if use_f32r else F32
    nc = bacc.Bacc("TRN2", target_bir_lowering=False, debug=False)

    emb = nc.dram_tensor("embeddings", [B_CORE, D], F32, kind="ExternalInput").ap()
    cen = nc.dram_tensor("centroids", [K, D], F32, kind="ExternalInput").ap()
    labels_out = nc.dram_tensor("labels", [TILES, P], I32, kind="ExternalOutput").ap()
    sums_out = nc.dram_tensor("sums", [1, 2 * K], F32, kind="ExternalOutput").ap()

    with tile.TileContext(nc) as tc, ExitStack() as ctx:
        const_pool = ctx.enter_context(tc.tile_pool(name="const", bufs=1))
        epool = ctx.enter_context(tc.tile_pool(name="e", bufs=2))
        etpool = ctx.enter_context(tc.tile_pool(name="et", bufs=GROUP + 2))
        sqpool = ctx.enter_context(tc.tile_pool(name="sq", bufs=2))
        e2pool = ctx.enter_context(tc.tile_pool(name="e2", bufs=6))
        tpool = ctx.enter_context(tc.tile_pool(name="t", bufs=2))
        distpool = ctx.enter_context(tc.tile_pool(name="dist", bufs=2))
        exwpool = ctx.enter_context(tc.tile_pool(name="exw", bufs=3))
        m8pool = ctx.enter_context(tc.tile_pool(name="m8", bufs=6))
        outpool = ctx.enter_context(tc.tile_pool(name="out", bufs=1))

        ps_et = ctx.enter_context(tc.tile_pool(name="ps_et", bufs=2, space="PSUM"))
        ps_dot = ctx.enter_context(tc.tile_pool(name="ps_dot", bufs=2, space="PSUM"))
        ps_sums = ctx.enter_context(tc.tile_pool(name="ps_sums", bufs=1, space="PSUM"))

        # ---------------- constants / centroid prep ----------------
        ident = const_pool.tile([P, P], F32)
        masks.make_identity(nc, ident[:])

        ones_bf = const_pool.tile([P, 1], BF16)
        nc.vector.memset(ones_bf[:], 1.0)
        shift_sb = const_pool.tile([P, 1], F32)
        nc.vector.memset(shift_sb[:], SHIFT)
        ones128 = const_pool.tile([P, P], F32)
        nc.vector.memset(ones128[:], 1.0)

        c_nat = const_pool.tile([P, 2, D], F32)  # two 128-row halves of C
        nc.sync.dma_start(c_nat[:], cen.rearrange("(h p) d -> p h d", h=2))

        # cT chunks: [d, k] with k = 256 across both halves
        ctsc = const_pool.tile([P, NCHUNK, K], mm_dt)   # 2 * C.T (rounded)
        ctsq = const_pool.tile([P, NCHUNK, K], F32)     # (C.T)^2
        for c in range(NCHUNK):
            pct = ps_dot.tile([P, 512], F32, tag="pd")
            for h in range(2):
                nc.tensor.transpose(
                    pct[:, h * P:(h + 1) * P],
                    c_nat[:, h, c * P:(c + 1) * P],
                    ident[:],
                )
            nc.scalar.activation(ctsc[:, c, :], pct[:, 0:K], AF.Copy, scale=2.0)
            nc.scalar.activation(ctsq[:, c, :], pct[:, 0:K], AF.Square)

        # c2 replicated across partitions: ones128.T @ ctsq  (fp32 matmuls)
        pc2 = ps_dot.tile([P, 512], F32, tag="pd")
        for c in range(NCHUNK):
            nc.tensor.matmul(pc2[:, 0:K], ones128[:], ctsq[:, c, :],
                             start=(c == 0), stop=(c == NCHUNK - 1))
        c2rep = const_pool.tile([P, K], F32)
        nc.vector.tensor_copy(c2rep[:], pc2[:, 0:K])

        labels_sb = const_pool.tile([P, TILES], F32)

        psum_s = ps_sums.tile([1, 2 * K], F32)

        emb_r = emb.rearrange("(g j p) d -> g p j d", g=NGROUPS, j=GROUP, p=P)

        # Software-pipelined (modulo) schedule: the sqrt/exp/w/labels/sums
        # tail of group g-1 is emitted at fixed slots inside group g's
        # emission stream so no engine's in-order queue ever blocks the
        # steady per-tile pipeline.
        def emit_sqrt_exp(prev):
            t_p, dist_p, exw_p, _, _ = prev
            nc.scalar.activation(dist_p[:], t_p[:], AF.Sqrt, scale=-1.0)
            nc.scalar.activation(exw_p[:, :, 0:K], dist_p[:],
                                 AF.Exp, bias=shift_sb[:], scale=-1.0)

        def emit_w(prev):
            _, dist_p, exw_p, _, _ = prev
            w_eng = nc.gpsimd if W_ON_GPSIMD else nc.vector
            w_eng.tensor_tensor(out=exw_p[:, :, K:2 * K], in0=dist_p[:],
                                in1=exw_p[:, :, 0:K], op=ALU.mult)

        def emit_labels(prev):
            _, _, _, i8_p, g_p = prev
            nc.vector.tensor_copy(
                labels_sb[:, g_p * GROUP:(g_p + 1) * GROUP], i8_p[:, :, 0])

        def emit_sums(prev):
            _, _, exw_p, _, g_p = prev
            for j in range(GROUP):
                idx = g_p * GROUP + j
                nc.tensor.matmul(psum_s[:], ones_bf[:], exw_p[:, j, :],
                                 start=(idx == 0), stop=(idx == TILES - 1),
                                 skip_group_check=True)

        prev = None   # tail state of group g-1
        prev2 = None  # tail state of group g-2 (for sums)

        # ---------------- main loop ----------------
        for g in range(NGROUPS):
            e_g = epool.tile([P, GROUP, D], F32)
            nc.sync.dma_start(e_g[:], emb_r[g])

            t_g = tpool.tile([P, GROUP, K], F32)
            dist_g = distpool.tile([P, GROUP, K], BF16)
            exw = exwpool.tile([P, GROUP, 2 * K], BF16)
            m8g = m8pool.tile([P, GROUP, 8], F32, tag="m8")
            i8g = m8pool.tile([P, GROUP, 8], U32, tag="i8")

            # ---- phase 1: e2 + transposes + PSUM->SBUF copies (batched so
            # the PE stays in transpose mode for the whole group) ----
            ets = []
            e2s = []
            for j in range(GROUP):
                ei = e_g[:, j, :]
                e2c = e2pool.tile([P, 1], F32)
                on_dve = E2_DVE_EVERY > 0 and j % E2_DVE_EVERY == E2_DVE_EVERY - 1
                if on_dve:
                    sqd = sqpool.tile([P, D], BF16, tag="sqv")
                    nc.vector.scalar_tensor_tensor(
                        out=sqd[:], in0=ei, scalar=1.0, in1=ei,
                        op0=ALU.mult, op1=ALU.mult, accum_out=e2c[:])
                else:
                    sqd = sqpool.tile([P, D], BF16, tag="sqa")
                    nc.scalar.activation(sqd[:], ei, AF.Square, accum_out=e2c[:])
                e2s.append(e2c)

                pet = ps_et.tile([P, D], F32)
                for c in range(NCHUNK):
                    nc.tensor.transpose(pet[:, c * P:(c + 1) * P],
                                        ei[:, c * P:(c + 1) * P], ident[:])
                et = etpool.tile([P, D], mm_dt)
                nc.scalar.copy(et[:, 0:512], pet[:, 0:512])
                nc.vector.tensor_copy(et[:, 512:D], pet[:, 512:D])
                ets.append(et)

            # group g-1's transcendentals go after phase 1's ACT ops so they
            # never delay the et copies the PE is waiting on
            if prev is not None:
                emit_sqrt_exp(prev)
                if W_ON_GPSIMD:
                    emit_w(prev)

            # ---- phase 2: dots + epilogue per tile ----
            pd = None
            for j in range(GROUP):
                et = ets[j]
                if j % 2 == 0:
                    pd = ps_dot.tile([P, 512], F32, tag="pd")
                half = (j % 2) * K
                for c in range(NCHUNK):
                    nc.tensor.matmul(pd[:, half:half + K],
                                     et[:, c * P:(c + 1) * P],
                                     ctsc[:, c, :],
                                     start=(c == 0), stop=(c == NCHUNK - 1))

                # t = (2*dot - e2) - c2 = -dist^2   (argmax_k t == argmin_k)
                tj = t_g[:, j, :]
                nc.vector.scalar_tensor_tensor(
                    out=tj, in0=pd[:, half:half + K], scalar=e2s[j][:],
                    in1=c2rep[:], op0=ALU.subtract, op1=ALU.subtract,
                )
                nc.vector.max(m8g[:, j, :], tj)
                nc.vector.max_index(i8g[:, j, :], m8g[:, j, :], tj)

                if j == 3 and prev is not None and not W_ON_GPSIMD:
                    emit_w(prev)
                if j == GROUP - 2 and prev2 is not None:
                    emit_sums(prev2)
                    prev2 = None

            if prev is not None:
                emit_labels(prev)
                prev2 = prev
            prev = (t_g, dist_g, exw, i8g, g)

        # ---- drain the pipeline for the last group(s) ----
        if prev2 is not None:
            emit_sums(prev2)
        if prev is not None:
            emit_sqrt_exp(prev)
            emit_w(prev)
            emit_labels(prev)
            emit_sums(prev)

        # ---------------- outputs ----------------
        sums_sb = outpool.tile([1, 2 * K], F32)
        nc.vector.tensor_copy(sums_sb[:], psum_s[:])
        nc.sync.dma_start(sums_out[:], sums_sb[:])

        plab = ps_dot.tile([P, 512], F32, tag="pd")
        nc.tensor.transpose(plab[:TILES, 0:P], labels_sb[:], ident[:])
        lab_i32 = outpool.tile([TILES, P], I32)
        nc.vector.tensor_copy(lab_i32[:], plab[:TILES, 0:P])
        nc.sync.dma_start(labels_out[:], lab_i32[:])

    nc.compile()
    return nc


_NC_CACHE = {}


def _get_nc(use_f32r: bool):
    if use_f32r not in _NC_CACHE:
        _NC_CACHE[use_f32r] = build_kernel(use_f32r)
    return _NC_CACHE[use_f32r]


def kernel(embeddings, centroids, _trace=False, _use_f32r=None):
    use_f32r = USE_F32R if _use_f32r is None else _use_f32r
    embeddings = np.ascontiguousarray(np.asarray(embeddings, dtype=np.float32))
    centroids = np.ascontiguousarray(np.asarray(centroids, dtype=np.float32))
    assert embeddings.shape == (B, D) and centroids.shape == (K, D)

    nc = _get_nc(use_f32r)
    in_maps = [
        {
            "embeddings": embeddings[i * B_CORE:(i + 1) * B_CORE],
            "centroids": centroids,
        }
        for i in range(N_CORES)
    ]
    res = run_bass_kernel_spmd(nc, in_maps, core_ids=list(range(N_CORES)),
                               trace=_trace)

    labels = np.concatenate(
        [res.results[i]["labels"].reshape(-1) for i in range(N_CORES)]
    ).astype(np.int32)

    sums = np.stack([res.results[i]["sums"][0] for i in range(N_CORES)])  # [8, 512]
    S = sums[:, 0:K].sum(axis=0)
    W = sums[:, K:2 * K].sum(axis=0)
    loss = np.float32(np.mean(W / S))

    if _trace:
        kernel._last_results = res
    return loss, labels


# revision 24
# speedup vs baseline: 1.1400x; 1.1388x over previous
"""Trainium2 Bass kernel for nn_ClusterLM (vq_codebook).

Computes, for embeddings E [65536, 768] and centroids C [256, 768]:
  dist[b,k]  = || E_b - C_k ||_2
  labels[b]  = argmin_k dist[b,k]
  loss       = mean_k( sum_b dist[b,k] * softmin_b(dist[:,k]) )

Data-parallel over 8 NeuronCores: each core handles 8192 rows of E, with
C replicated.  Per-centroid softmin statistics use a fixed exp-shift
(exp(SHIFT - dist)) so the per-core partial sums S_k = sum_b exp(SHIFT-d)
and W_k = sum_b d*exp(SHIFT-d) merge on the host with no collective.

Layout on core: [b, k] (batch rows on partitions, centroids on free dim).
  - E tiles [128, 768] are PE-transposed to eT [d, b] chunks for the
    d-contraction matmuls (out[b,k] = sum_d eT[d,b] * cT[d,k]).
  - cT holds 2*C.T so psum = 2*dot; t = 2*dot - c2 makes argmax_k(t) the
    argmin over distances, and dist = sqrt(e2 - t).
  - S/W column sums over the batch go through ones.T @ [ex|w] matmuls.
"""

import os
import sys

sys.path.insert(0, "/opt/trn_rl_repo")

from contextlib import ExitStack

import numpy as np

import concourse.bacc as bacc
import concourse.bass as bass
import concourse.mybir as mybir
import concourse.tile as tile
from concourse import masks
from concourse.bass_utils import run_bass_kernel_spmd

F32 = mybir.dt.float32
F32R = mybir.dt.float32r
BF16 = mybir.dt.bfloat16
I32 = mybir.dt.int32
U32 = mybir.dt.uint32
AF = mybir.ActivationFunctionType
ALU = mybir.AluOpType

P = 128          # partitions
D = 768          # embedding dim
K = 256          # centroids
NCHUNK = D // P  # 6 contraction chunks
N_CORES = 8
B = 65536
B_CORE = B // N_CORES       # 8192
TILES = B_CORE // P         # 64
GROUP = 8                   # tiles per epilogue group
NGROUPS = TILES // GROUP    # 8
SHIFT = 36.0                # exp(SHIFT - dist); dist in [33, 45] for randn data

USE_F32R = os.environ.get("VQ_USE_F32R", "1") == "1"
E2_DVE_EVERY = int(os.environ.get("VQ_E2_DVE_EVERY", "3"))  # 0=all ACT; n: every n-th tile on DVE
W_ON_GPSIMD = os.environ.get("VQ_W_GPS", "1") == "1"


def build_kernel(use_f32r: bool):
    mm_dt = F32R if use_f32r else F32
    nc = bacc.Bacc("TRN2", target_bir_lowering=False, debug=False)

    emb = nc.dram_tensor("embeddings", [B_CORE, D], F32, kind="ExternalInput").ap()
    cen = nc.dram_tensor("centroids", [K, D], F32, kind="ExternalInput").ap()
    labels_out = nc.dram_tensor("labels", [TILES, P], I32, kind="ExternalOutput").ap()
    sums_out = nc.dram_tensor("sums", [1, 2 * K], F32, kind="ExternalOutput").ap()

    with tile.TileContext(nc) as tc, ExitStack() as ctx:
        const_pool = ctx.enter_context(tc.tile_pool(name="const", bufs=1))
        epool = ctx.enter_context(tc.tile_pool(name="e", bufs=2))
        etpool = ctx.enter_context(tc.tile_pool(name="et", bufs=GROUP + 2))
        sqpool = ctx.enter_context(tc.tile_pool(name="sq", bufs=2))
        e2pool = ctx.enter_context(tc.tile_pool(name="e2", bufs=6))
        tpool = ctx.enter_context(tc.tile_pool(name="t", bufs=2))
        distpool = ctx.enter_context(tc.tile_pool(name="dist", bufs=2))
        exwpool = ctx.enter_context(tc.tile_pool(name="exw", bufs=3))
        m8pool = ctx.enter_context(tc.tile_pool(name="m8", bufs=6))
        outpool = ctx.enter_context(tc.tile_pool(name="out", bufs=1))

        ps_et = ctx.enter_context(tc.tile_pool(name="ps_et", bufs=2, space="PSUM"))
        ps_dot = ctx.enter_context(tc.tile_pool(name="ps_dot", bufs=2, space="PSUM"))
        ps_sums = ctx.enter_context(tc.tile_pool(name="ps_sums", bufs=1, space="PSUM"))

        # ---------------- constants / centroid prep ----------------
        ident = const_pool.tile([P, P], F32)
        masks.make_identity(nc, ident[:])

        ones_bf = const_pool.tile([P, 1], BF16)
        nc.vector.memset(ones_bf[:], 1.0)
        shift_sb = const_pool.tile([P, 1], F32)
        nc.vector.memset(shift_sb[:], SHIFT)
        ones128 = const_pool.tile([P, P], F32)
        nc.vector.memset(ones128[:], 1.0)

        c_nat = const_pool.tile([P, 2, D], F32)  # two 128-row halves of C
        nc.sync.dma_start(c_nat[:], cen.rearrange("(h p) d -> p h d", h=2))

        # cT chunks: [d, k] with k = 256 across both halves
        ctsc = const_pool.tile([P, NCHUNK, K], mm_dt)   # 2 * C.T (rounded)
        ctsq = const_pool.tile([P, NCHUNK, K], F32)     # (C.T)^2
        for c in range(NCHUNK):
            pct = ps_dot.tile([P, 512], F32, tag="pd")
            for h in range(2):
                nc.tensor.transpose(
                    pct[:, h * P:(h + 1) * P],
                    c_nat[:, h, c * P:(c + 1) * P],
                    ident[:],
                )
            nc.scalar.activation(ctsc[:, c, :], pct[:, 0:K], AF.Copy, scale=2.0)
            nc.scalar.activation(ctsq[:, c, :], pct[:, 0:K], AF.Square)

        # c2 replicated across partitions: ones128.T @ ctsq  (fp32 matmuls)
        pc2 = ps_dot.tile([P, 512], F32, tag="pd")
        for c in range(NCHUNK):
            nc.tensor.matmul(pc2[:, 0:K], ones128[:], ctsq[:, c, :],
                             start=(c == 0), stop=(c == NCHUNK - 1))
        c2rep = const_pool.tile([P, K], F32)
        nc.vector.tensor_copy(c2rep[:], pc2[:, 0:K])

        labels_sb = const_pool.tile([P, TILES], F32)

        psum_s0 = ps_sums.tile([1, 2 * K], F32, tag="s0")
        psum_s1 = ps_sums.tile([1, 2 * K], F32, tag="s1")
        psum_s = [psum_s0, psum_s1]

        emb_r = emb.rearrange("(g j p) d -> g p j d", g=NGROUPS, j=GROUP, p=P)

        # Software-pipelined (modulo) schedule: the sqrt/exp/w/labels/sums
        # tail of group g-1 is emitted at fixed slots inside group g's
        # emission stream so no engine's in-order queue ever blocks the
        # steady per-tile pipeline.
        def emit_sqrt_exp(prev):
            t_p, dist_p, exw_p, _, _ = prev
            nc.scalar.activation(dist_p[:], t_p[:], AF.Sqrt, scale=-1.0)
            nc.scalar.activation(exw_p[:, :, 0:K], dist_p[:],
                                 AF.Exp, bias=shift_sb[:], scale=-1.0)

        def emit_w(prev):
            _, dist_p, exw_p, _, _ = prev
            w_eng = nc.gpsimd if W_ON_GPSIMD else nc.vector
            w_eng.tensor_tensor(out=exw_p[:, :, K:2 * K], in0=dist_p[:],
                                in1=exw_p[:, :, 0:K], op=ALU.mult)

        def emit_labels(prev):
            _, _, _, i8_p, g_p = prev
            nc.vector.tensor_copy(
                labels_sb[:, g_p * GROUP:(g_p + 1) * GROUP], i8_p[:, :, 0])

        def emit_sums(prev):
            # ping-pong between two PSUM banks to dodge same-bank back-to-back
            # accumulation serialization; host-side merge sums both halves
            _, _, exw_p, _, g_p = prev
            for j in range(GROUP):
                idx = g_p * GROUP + j
                ps = psum_s[j % 2]
                nc.tensor.matmul(ps[:], ones_bf[:], exw_p[:, j, :],
                                 start=(idx < 2), stop=(idx >= TILES - 2),
                                 skip_group_check=True)

        prev = None   # tail state of group g-1
        prev2 = None  # tail state of group g-2 (for sums)

        # ---------------- main loop ----------------
        for g in range(NGROUPS):
            e_g = epool.tile([P, GROUP, D], F32)
            nc.sync.dma_start(e_g[:], emb_r[g])

            t_g = tpool.tile([P, GROUP, K], F32)
            dist_g = distpool.tile([P, GROUP, K], BF16)
            exw = exwpool.tile([P, GROUP, 2 * K], BF16)
            m8g = m8pool.tile([P, GROUP, 8], F32, tag="m8")
            i8g = m8pool.tile([P, GROUP, 8], U32, tag="i8")

            # ---- phase 1: e2 + transposes + PSUM->SBUF copies (batched so
            # the PE stays in transpose mode for the whole group).  DVE-side
            # e2 ops go AFTER the copies so they never delay the PE's dots.
            ets = []
            e2s = [None] * GROUP
            dve_e2 = []
            for j in range(GROUP):
                ei = e_g[:, j, :]
                on_dve = E2_DVE_EVERY > 0 and j % E2_DVE_EVERY == E2_DVE_EVERY - 1
                if on_dve:
                    dve_e2.append(j)
                else:
                    e2c = e2pool.tile([P, 1], F32)
                    sqd = sqpool.tile([P, D], BF16, tag="sqa")
                    nc.scalar.activation(sqd[:], ei, AF.Square, accum_out=e2c[:])
                    e2s[j] = e2c

                pet = ps_et.tile([P, D], F32)
                for c in range(NCHUNK):
                    nc.tensor.transpose(pet[:, c * P:(c + 1) * P],
                                        ei[:, c * P:(c + 1) * P], ident[:])
                et = etpool.tile([P, D], mm_dt)
                nc.scalar.copy(et[:, 0:512], pet[:, 0:512])
                nc.vector.tensor_copy(et[:, 512:D], pet[:, 512:D])
                ets.append(et)

            for j in dve_e2:
                e2c = e2pool.tile([P, 1], F32)
                sqd = sqpool.tile([P, D], BF16, tag="sqv")
                nc.vector.scalar_tensor_tensor(
                    out=sqd[:], in0=e_g[:, j, :], scalar=1.0, in1=e_g[:, j, :],
                    op0=ALU.mult, op1=ALU.mult, accum_out=e2c[:])
                e2s[j] = e2c

            # group g-1's transcendentals go after phase 1's ACT ops so they
            # never delay the et copies the PE is waiting on
            if prev is not None:
                emit_sqrt_exp(prev)
                if W_ON_GPSIMD:
                    emit_w(prev)

            # ---- phase 2: dots + epilogue per tile ----
            pd = None
            for j in range(GROUP):
                et = ets[j]
                if j % 2 == 0:
                    pd = ps_dot.tile([P, 512], F32, tag="pd")
                half = (j % 2) * K
                for c in range(NCHUNK):
                    nc.tensor.matmul(pd[:, half:half + K],
                                     et[:, c * P:(c + 1) * P],
                                     ctsc[:, c, :],
                                     start=(c == 0), stop=(c == NCHUNK - 1))

                # t = (2*dot - e2) - c2 = -dist^2   (argmax_k t == argmin_k)
                tj = t_g[:, j, :]
                nc.vector.scalar_tensor_tensor(
                    out=tj, in0=pd[:, half:half + K], scalar=e2s[j][:],
                    in1=c2rep[:], op0=ALU.subtract, op1=ALU.subtract,
                )
                nc.vector.max(m8g[:, j, :], tj)
                nc.vector.max_index(i8g[:, j, :], m8g[:, j, :], tj)

                if j == 3 and prev is not None and not W_ON_GPSIMD:
                    emit_w(prev)
                if j == GROUP - 2 and prev2 is not None:
                    emit_sums(prev2)
                    prev2 = None

            if prev is not None:
                emit_labels(prev)
                prev2 = prev
            prev = (t_g, dist_g, exw, i8g, g)

        # ---- drain the pipeline for the last group(s) ----
        if prev2 is not None:
            emit_sums(prev2)
        if prev is not None:
            emit_sqrt_exp(prev)
            emit_w(prev)
            emit_labels(prev)
            emit_sums(prev)

        # ---------------- outputs ----------------
        sums_sb = outpool.tile([1, 2 * K], F32)
        nc.vector.tensor_copy(sums_sb[:], psum_s[0][:])
        nc.vector.tensor_tensor(out=sums_sb[:], in0=sums_sb[:],
                                in1=psum_s[1][:], op=ALU.add)
        nc.sync.dma_start(sums_out[:], sums_sb[:])

        plab = ps_dot.tile([P, 512], F32, tag="pd")
        nc.tensor.transpose(plab[:TILES, 0:P], labels_sb[:], ident[:])
        lab_i32 = outpool.tile([TILES, P], I32)
        nc.vector.tensor_copy(lab_i32[:], plab[:TILES, 0:P])
        nc.sync.dma_start(labels_out[:], lab_i32[:])

    nc.compile()
    return nc


_NC_CACHE = {}


def _get_nc(use_f32r: bool):
    if use_f32r not in _NC_CACHE:
        _NC_CACHE[use_f32r] = build_kernel(use_f32r)
    return _NC_CACHE[use_f32r]


def kernel(embeddings, centroids, _trace=False, _use_f32r=None):
    use_f32r = USE_F32R if _use_f32r is None else _use_f32r
    embeddings = np.ascontiguousarray(np.asarray(embeddings, dtype=np.float32))
    centroids = np.ascontiguousarray(np.asarray(centroids, dtype=np.float32))
    assert embeddings.shape == (B, D) and centroids.shape == (K, D)

    nc = _get_nc(use_f32r)
    in_maps = [
        {
            "embeddings": embeddings[i * B_CORE:(i + 1) * B_CORE],
            "centroids": centroids,
        }
        for i in range(N_CORES)
    ]
    res = run_bass_kernel_spmd(nc, in_maps, core_ids=list(range(N_CORES)),
                               trace=_trace)

    labels = np.concatenate(
        [res.results[i]["labels"].reshape(-1) for i in range(N_CORES)]
    ).astype(np.int32)

    sums = np.stack([res.results[i]["sums"][0] for i in range(N_CORES)])  # [8, 512]
    S = sums[:, 0:K].sum(axis=0)
    W = sums[:, K:2 * K].sum(axis=0)
    loss = np.float32(np.mean(W / S))

    if _trace:
        kernel._last_results = res
    return loss, labels
